# revision 29
# baseline (speedup 1.0000x reference)
"""AdderNet 2D conv (L1-distance "convolution") on 8 TRN2 NeuronCores.

Reference computation:
    X_col = unfold(x, k=3, stride=1, pad=1)      # (N, D, P)  D=576, P=196
    out[n, f, p] = -sum_d |W_col[f, d] - X_col[n, d, p]|

Distribution: filter-parallel - core i computes filters f in [8i, 8i+8)
for the FULL batch (no collectives; host concatenates filter slices).

Identity used on-device:
  -sum_d |x-w|  =  (-sum_d x) + (sum_d w) + 2*sum_d min(x-w, 0)
The rank-1 terms (-sum_d x per position, +sum_d w per filter) are
added ON THE HOST after the gather, so the device does ONLY the
min/relu elementwise stream + PE reduction.

Per-core pipeline (raw Bass):
  - Host im2col, d (patch dim, 576, (kh,kw,c)-ordered) on SBUF
    partitions: FOUR full 128-row chunks plus one FOLDED half-chunk
    (d 512:576 for positions 0:1568 on partitions 0:64 and positions
    1568:3136 on partitions 64:128 -> a (128, 1568) tile).
  - Positions are i-MAJOR (p = i*224 + n*14 + j), which makes the
    padding zeros of the kh=0 rows (chunk d 0:128) the FIRST 224
    positions and of the kh=2 rows (chunk d 384:512) the LAST 224;
    those column ranges are skipped on-device (about 5% less
    elementwise + DMA) and the host adds the constant
    2*sum min(-w,0) to the border position blocks.
  - Input DMAs: W columns (fp32, values pre-rounded to bf16 so
    device and host arithmetic agree) ride the SCALAR engine's own
    DMA ring (parallel latency); the sync ring carries, in
    consumption order, the folded tile as a 392-col piece + rest
    (first elementwise op starts ~9.9us), then chunks 1..4 with the
    skipped border columns trimmed.  NOT the gpsimd queue: its SWDGE
    path measured ~5us issue-to-data vs ~2us for hardware rings.
  - Per (filter, chunk) unit, ONE elementwise instruction:
      VectorE: tensor_scalar(op0=sub W[f,.], op1=min 0) -> min(x-w,0)
        (4x_2p DVE perf mode; ~946ns/full 3136-col unit)
      ScalarE: activation(Relu, scale=-1, bias=W[f,.])  -> relu(w-x)
        (1x rate, ~2.9us/full unit; capped by HW at 1 elem/cyc/lane)
    Split by measured rates: ScalarE gets filters {3,7} in the folded
    chunk and chunks 1-3, filter 3 in chunk 4, plus cols 1176:2912 of
    the chunk-4 filter-5/7 units (VectorE makes cols 0:1176).  The
    LAST unit (filter 6, chunk 4) is emitted as per-stream quarter
    ops in order (0,1,3,2) so stream completion cascades into the
    output path.  GPSIMD is useless here: its tensor_scalar measured
    45x slower than DVE and TT-min / scalar_tensor_tensor fail the
    Pool-engine codegen check.
  - TensorE reduces over partitions into PSUM, 4-way column-tiled:
    stream c (tile_position=(0,32c)) computes ALL 8 filters for its
    private position quarter [784c, 784c+784), psum rows 32c..32c+8,
    banks {2c, 2c+1}.  Stationary blocks: +/-2 (sign by producing
    engine; ST_XTRA/ST_XTRA2 hold the sign-flipped variants for the
    split units).  start=True clears has_written BANK-wide, so only
    the FIRST matmul touching a (stream, bank) carries it - that also
    clears the HAM-warmup garbage (no separate init pass).
  - HAM warmup: dummy matmuls (on possibly-uninitialized operands -
    results are discarded) keep the PE busy from engine boot through
    the input-DMA wait so its clock gate opens (1.2 -> 2.4 GHz).
  - Tail: plain PSUM->SBUF copies into a bf16 staging tile (host
    upcasts; well inside the error budget): ScalarE evacuates
    streams 0, 1 and the right half of 2; VectorE stream 3 then the
    left half of 2 (consecutive DVE PSUM ops chain fine - the big
    DRAIN overlaps the next op).  Output DMAs spread over queues:
    sync ships d0, d3; gpsimd d1; ScalarE HWDGE d2; the output rings
    are pre-warmed by a tiny dummy store at ~11us.  Host adds
    sum_d W[f] - sum_d x[p] + border constants and transposes
    stream-major i-major -> (N, F, H, W).  NOTE: ordering any
    per-stream quarter op BETWEEN the chunk-4 split parts (to fire
    its stream sem early) reproducibly HANGS the device - likely the
    cayman event-accel deadlock; quarters must stay at the very end.

kernel(x, W) accepts the FULL inputs and returns the FULL output.
"""

import numpy as np
import ml_dtypes

import concourse.bass as bass
from concourse import mybir
from concourse.bass_utils import run_bass_kernel_spmd

# Problem constants (hardcoded per harness rules)
N, C, H, W_SP = 16, 64, 14, 14
F = 64
KK = 3
PAD = 1
P = H * W_SP            # 196 output positions per image
POS = N * P             # 3136 total positions
D = C * KK * KK         # 576
N_CORES = 8
F_PER = F // N_CORES    # 8 filters per core
NFULL = 4               # full 128-row d-chunks (d 0:512)
FOLD_FD = POS // 2      # 1568: folded chunk free dim
FQ1 = 392               # first folded DMA piece (cols 0:392)
NSTREAM = 4             # TensorE column-tiling streams
QPOS = POS // NSTREAM   # 784 positions per stream (its private quarter)
BANKC = 512             # psum bank capacity in f32
RING_V = 8              # vector-produced tile ring
RING_S = 4              # scalar-produced tile ring
N_WARM = 7              # HAM-warmup dummy matmul rounds per stream

FP32 = mybir.dt.float32
BF16 = mybir.dt.bfloat16

WCOLS = (NFULL + 1) * F_PER  # 40 W columns (full: k*8+j; folded: 32+j)
# stat layout (bf16): [0:64) full blocks (8 per filter, col j = +/-2
# by that filter's engine), [64:128) folded TOP blocks (rows 0:64),
# [128:192) folded BOT blocks (rows 64:128), [192:200) always-zero
# block for the HAM-warmup dummies, [200:208) the +2 full-variant
# block for the DVE half of the split unit (filter 7 is ACT -> -2
# in ST_FULL, but its chunk-4 DVE half needs +2).
ST_FULL = 0
ST_TOP = 64
ST_BOT = 128
ST_ZERO = 192
ST_XTRA = 200
ST_XTRA2 = 208
ST_N = 216
ACT_F = (3, 7)          # ScalarE filters (folded + chunks 1..3)
SPLIT_V = 1176          # chunk-4 filters 5,7: DVE cols 0:1176, ACT rest
LAST_J = 6              # chunk-4 filter 6: quartered per stream
# i-major position ordering (p = i*224 + n*14 + j) makes the padding
# zeros of kh=0 rows (chunk d[0:128]) the FIRST 224 positions and of
# kh=2 rows (chunk d[384:512]) the LAST 224: those column ranges are
# skipped on-device (min(0-w,0) is a per-filter constant the host
# adds to the border blocks).
BORD = 224
CH_LO = {1: BORD, 2: 0, 3: 0, 4: 0}            # per-chunk valid cols
CH_HI = {1: POS, 2: POS, 3: POS, 4: POS - BORD}


def _pieces(k, a, b):
    """(stream, psum_lo, mov_lo, width) pieces for source cols [a,b).

    Full chunks (k>=1): position p = col; stream c covers positions
    [784c, 784c+784).  Folded chunk (k==0): cols 0:784 are positions
    0:1568 halves -> streams 0 (TOP rows) & 2 (BOT rows); cols
    784:1568 -> streams 1 & 3.  psum col = position within quarter.
    """
    out = []
    if k == 0:
        lo, hi = a, min(b, QPOS)
        if lo < hi:
            out += [(0, lo, lo - a, hi - lo), (2, lo, lo - a, hi - lo)]
        lo, hi = max(a, QPOS), b
        if lo < hi:
            out += [(1, lo - QPOS, lo - a, hi - lo),
                    (3, lo - QPOS, lo - a, hi - lo)]
    else:
        for c in range(NSTREAM):
            lo, hi = max(a, QPOS * c), min(b, QPOS * (c + 1))
            if lo < hi:
                out.append((c, lo - QPOS * c, lo - a, hi - lo))
    return out


def _sub_split(lo, width):
    """Split a psum col range at the 512 bank boundary."""
    segs = []
    hi = lo + width
    if lo < BANKC and hi > BANKC:
        segs.append((lo, BANKC - lo))
        segs.append((BANKC, hi - BANKC))
    else:
        segs.append((lo, width))
    return segs


def build_bass():
    nc = bass.Bass()

    w_ext = nc.declare_dram_parameter("wcols", [128, WCOLS], FP32,
                                      isOutput=False)
    xf_ext = nc.declare_dram_parameter("xfold", [128, FOLD_FD], BF16,
                                       isOutput=False)
    x_ext = nc.declare_dram_parameter("xcol", [NFULL, 128, POS], BF16,
                                      isOutput=False)
    out_ext = nc.declare_dram_parameter("out", [NSTREAM, F_PER, QPOS],
                                        BF16, isOutput=True)

    # SBUF
    w_sb = nc.alloc_sbuf_tensor("w_sb", [128, WCOLS], FP32)
    stat = nc.alloc_sbuf_tensor("stat", [128, ST_N], BF16)
    zmov = nc.alloc_sbuf_tensor("zmov", [128, BANKC], BF16)
    xfold = nc.alloc_sbuf_tensor("xfold_sb", [128, FOLD_FD], BF16)
    xch = [nc.alloc_sbuf_tensor(f"xc{k}", [128, POS], BF16)
           for k in range(NFULL)]
    vring = [nc.alloc_sbuf_tensor(f"vb{r}", [128, POS], BF16)
             for r in range(RING_V)]
    sring = [nc.alloc_sbuf_tensor(f"sb{r}", [128, POS], BF16)
             for r in range(RING_S)]
    osb = nc.alloc_sbuf_tensor("osb", [128, QPOS], BF16)

    # PSUM: 8 banks; stream c owns banks {2c, 2c+1} = cols
    # [1024c, 1024c+784).
    psum = nc.alloc_psum_tensor("ps", [128, 8 * BANKC], FP32)

    # ---- op list, in PE consumption order --------------------------
    ops = []

    def add_op(eng, j, k, s_c0, fd, xtra=False):
        ops.append(dict(eng=eng, j=j, k=k, s_c0=s_c0, fd=fd, xtra=xtra,
                        pieces=_pieces(k, s_c0, s_c0 + fd)))

    VF = [j for j in range(F_PER) if j not in ACT_F]
    # folded chunk, first piece (cols 0:392) then rest (392:1568)
    for (a, b) in ((0, FQ1), (FQ1, FOLD_FD)):
        for j in VF:
            add_op("v", j, 0, a, b - a)
        for j in ACT_F:
            add_op("s", j, 0, a, b - a)
    # full chunks 1..3: vector filters first, then scalar
    for k in (1, 2, 3):
        for j in VF:
            add_op("v", j, k, CH_LO[k], CH_HI[k] - CH_LO[k])
        for j in ACT_F:
            add_op("s", j, k, CH_LO[k], CH_HI[k] - CH_LO[k])
    # chunk 4: vector filters (minus LAST_J and split 5), scalar
    # filter 3, the asymmetric filter-5/7 splits (DVE 0:1176, ACT
    # rest), then the quartered LAST_J unit
    for j in [j for j in VF if j not in (LAST_J, 5)]:
        add_op("v", j, 4, 0, CH_HI[4])
    add_op("s", 3, 4, 0, CH_HI[4])
    add_op("v", 5, 4, 0, SPLIT_V)                    # +2 via ST_FULL
    add_op("v", 7, 4, 0, SPLIT_V, xtra=True)         # +2 via ST_XTRA
    add_op("s", 5, 4, SPLIT_V, CH_HI[4] - SPLIT_V, xtra=True)  # ST_XTRA2
    add_op("s", 7, 4, SPLIT_V, CH_HI[4] - SPLIT_V)   # -2 via ST_FULL
    for c in (0, 1, 3, 2):
        add_op("v", LAST_J, 4, QPOS * c,
               min(QPOS * (c + 1), CH_HI[4]) - QPOS * c)

    # producer indices
    nv = ns = 0
    for op in ops:
        if op["eng"] == "v":
            op["r"] = nv
            nv += 1
        else:
            op["r"] = ns
            ns += 1

    # last op touching each stream (for stsem + stop flags)
    last_for_stream = {}
    for idx, op in enumerate(ops):
        for (c, _pl, _ml, _wd) in op["pieces"]:
            last_for_stream[c] = idx

    def stat_block(op, c):
        if op["xtra"]:
            base = ST_XTRA if op["eng"] == "v" else ST_XTRA2
            return stat[:, base:base + 8]
        j = op["j"]
        if op["k"] == 0:
            base = ST_TOP if c < 2 else ST_BOT
        else:
            base = ST_FULL
        return stat[:, base + 8 * j:base + 8 * j + 8]

    with (
        nc.Block() as block,
        nc.semaphore("xw_sem") as xw_sem,
        nc.semaphore("xfq_sem") as xfq_sem,    # folded piece 1 (gp)
        nc.semaphore("xfr_sem") as xfr_sem,    # folded rest (sync)
        nc.semaphore("x0_sem") as x0_sem,
        nc.semaphore("x1_sem") as x1_sem,
        nc.semaphore("x2_sem") as x2_sem,
        nc.semaphore("x3_sem") as x3_sem,
        nc.semaphore("out_sem") as out_sem,
        nc.semaphore("stat_sem") as stat_sem,  # stat blocks written
        nc.semaphore("dve_sem") as dve_sem,
        nc.semaphore("actp_sem") as actp_sem,
        nc.semaphore("pe_v_sem") as pe_v_sem,
        nc.semaphore("pe_s_sem") as pe_s_sem,
        nc.semaphore("ev01_sem") as ev01_sem,  # ScalarE evacs done
        nc.semaphore("ev23_sem") as ev23_sem,  # VectorE evacs done
        nc.semaphore("warm_sem") as warm_sem,  # ring pre-warm stores
        nc.semaphore("st0_sem") as st0_sem,
        nc.semaphore("st1_sem") as st1_sem,
        nc.semaphore("st2_sem") as st2_sem,
        nc.semaphore("st3_sem") as st3_sem,
    ):
        xsem = {1: x0_sem, 2: x1_sem, 3: x2_sem, 4: x3_sem}
        stsem = [st0_sem, st1_sem, st2_sem, st3_sem]

        def chunk_wait(eng_q, op, seen):
            k = op["k"]
            if k == 0:
                sem = xfq_sem if op["s_c0"] < FQ1 else xfr_sem
                if ("fold", op["s_c0"] < FQ1) not in seen:
                    seen[("fold", op["s_c0"] < FQ1)] = True
                    eng_q.wait_ge(sem, 16)
            else:
                if k not in seen:
                    seen[k] = True
                    eng_q.wait_ge(xsem[k], 16)

        @block.sync
        def _(sync: bass.BassEngine):
            # input DMAs in consumption order (fold piece 1, fold
            # rest, chunks); W rides the scalar engine's ring.
            sync.dma_start(out=xfold[:, 0:FQ1], in_=xf_ext[:, 0:FQ1],
                           single_packet=True).then_inc(xfq_sem, 16)
            sync.dma_start(out=xfold[:, FQ1:FOLD_FD],
                           in_=xf_ext[:, FQ1:FOLD_FD],
                           single_packet=True).then_inc(xfr_sem, 16)
            for k in range(NFULL):
                lo, hi = CH_LO[k + 1], CH_HI[k + 1]
                sync.dma_start(out=xch[k][:, lo:hi],
                               in_=x_ext[k][:, lo:hi],
                               single_packet=True).then_inc(xsem[k + 1], 16)
            # pre-warm the output ring (tiny store, overwritten later)
            sync.dma_start(out=out_ext[0][0:1, 0:2],
                           in_=osb[0:1, 0:2]).then_inc(warm_sem, 16)
            # output stores: d0, d1 (ScalarE evacs), d3 (VectorE evac)
            # fresh ring warm just before the tail (the boot-time warm
            # is ~27us stale by output time; descriptor-fetch latency
            # may be recency-sensitive)
            sync.wait_ge(dve_sem, 32)
            sync.dma_start(out=out_ext[0][0:1, 4:6],
                           in_=osb[0:1, 4:6]).then_inc(warm_sem, 16)
            sync.wait_ge(ev01_sem, 1)
            sync.dma_start(out=out_ext[0], in_=osb[0:F_PER, :],
                           single_packet=True).then_inc(out_sem, 16)
            sync.wait_ge(ev23_sem, 1)
            sync.dma_start(out=out_ext[3], in_=osb[96:96 + F_PER, :],
                           single_packet=True).then_inc(out_sem, 16)
            sync.wait_ge(ev01_sem, 3)
            sync.wait_ge(ev23_sem, 2)
            sync.dma_start(out=out_ext[2], in_=osb[64:64 + F_PER, :],
                           single_packet=True).then_inc(out_sem, 16)
            sync.wait_ge(out_sem, 16 * NSTREAM)

        @block.vector
        def _(vector: bass.BassEngine):
            vector.wait_ge(xw_sem, 16)
            seen = {}
            for op in ops:
                if op["eng"] != "v":
                    continue
                r = op["r"]
                chunk_wait(vector, op, seen)
                if r >= RING_V:
                    vector.wait_ge(pe_v_sem, r - RING_V + 1)
                col = (32 + op["j"]) if op["k"] == 0 \
                    else ((op["k"] - 1) * F_PER + op["j"])
                src = xfold if op["k"] == 0 else xch[op["k"] - 1]
                c0 = op["s_c0"]
                vector.tensor_scalar(
                    out=vring[r % RING_V][:, 0:op["fd"]],
                    in0=src[:, c0:c0 + op["fd"]],
                    scalar1=w_sb[:, col:col + 1], scalar2=0.0,
                    op0=mybir.AluOpType.subtract,
                    op1=mybir.AluOpType.min,
                ).then_inc(dve_sem, 1)
            # evacuate stream 3, then the left half of stream 2 (the
            # right half runs in parallel on ScalarE)
            vector.wait_ge(stsem[3], 1)
            vector.tensor_scalar(
                out=osb[96:96 + F_PER, :],
                in0=psum[96:96 + F_PER, 3072:3072 + QPOS],
                scalar1=0.0, scalar2=None,
                op0=mybir.AluOpType.add,
            ).then_inc(ev23_sem, 1)
            vector.wait_ge(stsem[2], 1)
            vector.tensor_scalar(
                out=osb[64:64 + F_PER, 0:392],
                in0=psum[64:64 + F_PER, 2048:2048 + 392],
                scalar1=0.0, scalar2=None,
                op0=mybir.AluOpType.add,
            ).then_inc(ev23_sem, 1)

        @block.scalar
        def _(scalar: bass.BassEngine):
            # W columns ride this engine's own DMA ring (parallel
            # latency with the sync ring's folded-x pieces)
            scalar.dma_start(out=w_sb[:], in_=w_ext[:],
                             single_packet=True).then_inc(xw_sem, 16)
            # touch the Relu table so the one-time ACT table load
            # overlaps the input DMAs instead of the first real unit
            scalar.activation(osb[0:1, 0:1], zmov[0:1, 0:1],
                              mybir.ActivationFunctionType.Relu,
                              bias=0.0, scale=1.0)
            scalar.wait_ge(xw_sem, 16)
            seen = {}
            for op in ops:
                if op["eng"] != "s":
                    continue
                r = op["r"]
                chunk_wait(scalar, op, seen)
                if r >= RING_S:
                    scalar.wait_ge(pe_s_sem, r - RING_S + 1)
                col = (32 + op["j"]) if op["k"] == 0 \
                    else ((op["k"] - 1) * F_PER + op["j"])
                src = xfold if op["k"] == 0 else xch[op["k"] - 1]
                c0 = op["s_c0"]
                scalar.activation(
                    sring[r % RING_S][:, 0:op["fd"]],
                    src[:, c0:c0 + op["fd"]],
                    mybir.ActivationFunctionType.Relu,
                    bias=w_sb[:, col:col + 1], scale=-1.0,
                ).then_inc(actp_sem, 1)
            # evacuate streams 0, 1 (st0 fires early thanks to the
            # q0-before-parts ordering), then the right half of
            # stream 2 (left half runs in parallel on VectorE), then
            # ship d2 from this queue
            for c in (0, 1):
                scalar.wait_ge(stsem[c], 1)
                scalar.activation(
                    osb[32 * c:32 * c + F_PER, :],
                    psum[32 * c:32 * c + F_PER, 1024 * c:1024 * c + QPOS],
                    mybir.ActivationFunctionType.Identity,
                    bias=0.0, scale=1.0,
                ).then_inc(ev01_sem, 1)
            scalar.wait_ge(stsem[2], 1)
            scalar.activation(
                osb[64:64 + F_PER, 392:QPOS],
                psum[64:64 + F_PER, 2048 + 392:2048 + QPOS],
                mybir.ActivationFunctionType.Identity,
                bias=0.0, scale=1.0,
            ).then_inc(ev01_sem, 1)

        @block.gpsimd
        def _(gp: bass.BassEngine):
            # one-time memsets (stationary blocks); warmup no longer
            # depends on them - it reads garbage and is discarded
            gp.memset(stat[:], 0.0)
            for j in range(F_PER):
                val = -2.0 if j in ACT_F else 2.0
                gp.memset(stat[:, ST_FULL + 8 * j + j:
                               ST_FULL + 8 * j + j + 1], val)
                gp.memset(stat[0:64, ST_TOP + 8 * j + j:
                               ST_TOP + 8 * j + j + 1], val)
                gp.memset(stat[64:128, ST_BOT + 8 * j + j:
                               ST_BOT + 8 * j + j + 1], val)
            # +2 full-variant block for filter 7's DVE split part and
            # -2 full-variant block for filter 5's ACT split part
            gp.memset(stat[:, ST_XTRA + 7:ST_XTRA + 8], 2.0)
            gp.memset(stat[:, ST_XTRA2 + 5:ST_XTRA2 + 6],
                      -2.0).then_inc(stat_sem, 1)
            # pre-warm this engine's output DMA ring
            gp.dma_start(out=out_ext[0][0:1, 2:4],
                         in_=osb[0:1, 2:4]).then_inc(warm_sem, 16)
            # fresh ring warm, then output store d1 (ScalarE's
            # second evacuation)
            gp.wait_ge(actp_sem, 11)
            gp.dma_start(out=out_ext[0][0:1, 6:8],
                         in_=osb[0:1, 6:8]).then_inc(warm_sem, 16)
            gp.wait_ge(ev01_sem, 2)
            gp.dma_start(out=out_ext[1], in_=osb[32:32 + F_PER, :],
                         single_packet=True).then_inc(out_sem, 16)

        @block.tensor
        def _(tensor: bass.BassEngine):
            # HAM warmup: keep the PE array busy through the input-DMA
            # wait so its clock gate opens (1.2 -> 2.4 GHz) before real
            # work.  Both operands may still be UNINITIALIZED - the
            # results are discarded (start=True'd by the first real
            # matmul into each region), so garbage is fine.
            for _w in range(N_WARM):
                for c in range(NSTREAM):
                    tensor.matmul(
                        psum[32 * c:32 * c + F_PER,
                             1024 * c:1024 * c + BANKC],
                        stat[:, ST_ZERO:ST_ZERO + 8],
                        zmov[:, 0:BANKC],
                        start=True, stop=True, skip_group_check=True,
                        tile_position=(0, 32 * c),
                    )
            tensor.wait_ge(stat_sem, 1)  # stationary blocks written
            started = set()
            for idx, op in enumerate(ops):
                kind, r = op["eng"], op["r"]
                if kind == "v":
                    tensor.wait_ge(dve_sem, r + 1)
                    a = vring[r % RING_V]
                else:
                    tensor.wait_ge(actp_sem, r + 1)
                    a = sring[r % RING_S]
                np_ = len(op["pieces"])
                for pi, (c, plo, mlo, wd) in enumerate(op["pieces"]):
                    blk = stat_block(op, c)
                    is_last_c = last_for_stream[c] == idx
                    segs = _sub_split(plo, wd)
                    for si, (so, sw) in enumerate(segs):
                        fin = pi == np_ - 1 and si == len(segs) - 1
                        # start=True clears has_written BANK-wide, so
                        # only the FIRST matmul touching a (stream,
                        # bank) carries it; later first-writes to other
                        # columns of that bank replace-on-write because
                        # the bank clear already reset their bits.
                        key = (c, 0 if so < BANKC else 1)
                        st = key not in started
                        started.add(key)
                        stop_f = is_last_c and si == len(segs) - 1
                        mm = tensor.matmul(
                            psum[32 * c:32 * c + F_PER,
                                 1024 * c + so:1024 * c + so + sw],
                            blk,
                            a[:, mlo + (so - plo):mlo + (so - plo) + sw],
                            start=st, stop=stop_f,
                            skip_group_check=True,
                            tile_position=(0, 32 * c),
                        )
                        if stop_f:
                            mm.then_inc(stsem[c], 1)
                        elif fin:
                            mm.then_inc(
                                pe_v_sem if kind == "v" else pe_s_sem, 1)

    return nc


def _prep_inputs(x: np.ndarray, W: np.ndarray):
    x = np.asarray(x, dtype=np.float32)
    W = np.asarray(W, dtype=np.float32)
    # Host im2col in (kh, kw, c) d-order, i-MAJOR positions
    # (p = i*224 + n*14 + j)
    xp = np.zeros((C, N, H + 2, W_SP + 2), np.float32)
    xp[:, :, PAD:PAD + H, PAD:PAD + W_SP] = x.transpose(1, 0, 2, 3)
    xc = np.zeros((D, POS), np.float32)
    for b in range(KK * KK):
        kh, kw = divmod(b, KK)
        xc[64 * b:64 * (b + 1), :] = (
            xp[:, :, kh:kh + H, kw:kw + W_SP]
            .transpose(0, 2, 1, 3).reshape(C, POS))
    xfull = (xc[:512].reshape(NFULL, 128, POS)).astype(ml_dtypes.bfloat16)
    xfoldb = xc[512:].astype(ml_dtypes.bfloat16)
    xfold = np.zeros((128, FOLD_FD), ml_dtypes.bfloat16)
    xfold[0:64, :] = xfoldb[:, 0:FOLD_FD]
    xfold[64:128, :] = xfoldb[:, FOLD_FD:POS]
    # -sum_d x over the bf16-ROUNDED values the device actually uses
    negx = -np.asarray(xfull, np.float32).sum(axis=(0, 1)) \
        - np.asarray(xfoldb, np.float32).sum(axis=0)
    # W_col in (kh, kw, c) d-order, bf16-rounded (shipped as fp32 so
    # tensor_scalar's scalar1 constraint is met, but the VALUES match
    # bf16 so device/host arithmetic agrees): (F, 576)
    Wp = W.transpose(0, 2, 3, 1).reshape(F, KK * KK * C)
    Wpb = np.asarray(Wp.astype(ml_dtypes.bfloat16), np.float32)
    sw = Wpb.sum(axis=1)  # (F,)
    wtiles = []
    for i in range(N_CORES):
        wt = np.zeros((128, WCOLS), np.float32)
        for k in range(NFULL):
            wt[:, k * F_PER:(k + 1) * F_PER] = \
                Wpb[F_PER * i:F_PER * (i + 1), 128 * k:128 * (k + 1)].T
        fb = Wpb[F_PER * i:F_PER * (i + 1), 512:D].T  # (64, 8)
        wt[0:64, 32:40] = fb
        wt[64:128, 32:40] = fb
        wtiles.append(wt)
    # skipped border contributions: 2*sum min(-w,0) over the skipped
    # d rows, added by the host to the border position blocks
    b0 = 2.0 * np.minimum(-Wpb[:, 0:128], 0.0).sum(axis=1)     # (F,)
    b13 = 2.0 * np.minimum(-Wpb[:, 384:512], 0.0).sum(axis=1)  # (F,)
    return xfull, xfold, wtiles, sw, negx, b0, b13


_CACHED_NC = None
LAST_RESULT = None  # BassKernelResults of the most recent run (for test.py)


def kernel(x: np.ndarray, W: np.ndarray, _trace: bool = False) -> np.ndarray:
    global _CACHED_NC, LAST_RESULT
    xfull, xfold, wtiles, sw, negx, b0, b13 = _prep_inputs(x, W)
    if _CACHED_NC is None:
        _CACHED_NC = build_bass()
    nc = _CACHED_NC
    in_maps = [{"xcol": xfull, "xfold": xfold, "wcols": wtiles[i]}
               for i in range(N_CORES)]
    res = run_bass_kernel_spmd(nc, in_maps, core_ids=list(range(N_CORES)),
                               trace=_trace)
    LAST_RESULT = res
    outs = [np.asarray(res.results[i]["out"], dtype=np.float32)
            .transpose(1, 0, 2).reshape(F_PER, POS)
            for i in range(N_CORES)]
    o = np.concatenate(outs, axis=0)                    # (64, 3136)
    # host finish: rank-1 identity terms + skipped border constants
    o = o + sw[:, None] + negx[None, :]
    o[:, 0:BORD] += b0[:, None]
    o[:, POS - BORD:POS] += b13[:, None]
    # i-major positions: p = i*224 + n*14 + j
    o = (o.reshape(F, H, N, W_SP).transpose(2, 0, 1, 3)
          .reshape(N, F, H, W_SP).astype(np.float32))
    return o


# revision 30
# speedup vs baseline: 1.0053x; 1.0053x over previous
"""AdderNet 2D conv (L1-distance "convolution") on 8 TRN2 NeuronCores.

Reference computation:
    X_col = unfold(x, k=3, stride=1, pad=1)      # (N, D, P)  D=576, P=196
    out[n, f, p] = -sum_d |W_col[f, d] - X_col[n, d, p]|

Distribution: filter-parallel - core i computes filters f in [8i, 8i+8)
for the FULL batch (no collectives; host concatenates filter slices).

Identity used on-device:
  -sum_d |x-w|  =  (-sum_d x) + (sum_d w) + 2*sum_d min(x-w, 0)
The rank-1 terms (-sum_d x per position, +sum_d w per filter) are
added ON THE HOST after the gather, so the device does ONLY the
min/relu elementwise stream + PE reduction.

Per-core pipeline (raw Bass):
  - Host im2col, d (patch dim, 576, (kh,kw,c)-ordered) on SBUF
    partitions: FOUR full 128-row chunks plus one FOLDED half-chunk
    (d 512:576 for positions 0:1568 on partitions 0:64 and positions
    1568:3136 on partitions 64:128 -> a (128, 1568) tile).
  - Positions are i-MAJOR (p = i*224 + n*14 + j), which makes the
    padding zeros of the kh=0 rows (chunk d 0:128) the FIRST 224
    positions and of the kh=2 rows (chunk d 384:512) the LAST 224;
    those column ranges are skipped on-device (about 5% less
    elementwise + DMA) and the host adds the constant
    2*sum min(-w,0) to the border position blocks.
  - Input DMAs: W columns (fp32, values pre-rounded to bf16 so
    device and host arithmetic agree) ride the SCALAR engine's own
    DMA ring (parallel latency); the sync ring carries, in
    consumption order, the folded tile as a 392-col piece + rest
    (first elementwise op starts ~9.9us), then chunks 1..4 with the
    skipped border columns trimmed.  NOT the gpsimd queue: its SWDGE
    path measured ~5us issue-to-data vs ~2us for hardware rings.
  - Per (filter, chunk) unit, ONE elementwise instruction:
      VectorE: tensor_scalar(op0=sub W[f,.], op1=min 0) -> min(x-w,0)
        (4x_2p DVE perf mode; ~946ns/full 3136-col unit)
      ScalarE: activation(Relu, scale=-1, bias=W[f,.])  -> relu(w-x)
        (1x rate, ~2.9us/full unit; capped by HW at 1 elem/cyc/lane)
    Split by measured rates: ScalarE gets filters {3,7} in the folded
    chunk and chunks 1-3, filter 3 in chunk 4, plus cols 1176:2912 of
    the chunk-4 filter-5/7 units (VectorE makes cols 0:1176).  The
    LAST unit (filter 6, chunk 4) is emitted as per-stream quarter
    ops in order (0,1,3,2) so stream completion cascades into the
    output path.  GPSIMD is useless here: its tensor_scalar measured
    45x slower than DVE and TT-min / scalar_tensor_tensor fail the
    Pool-engine codegen check.
  - TensorE reduces over partitions into PSUM, 4-way column-tiled:
    stream c (tile_position=(0,32c)) computes ALL 8 filters for its
    private position quarter [784c, 784c+784), psum rows 32c..32c+8,
    banks {2c, 2c+1}.  Stationary blocks: +/-2 (sign by producing
    engine; ST_XTRA/ST_XTRA2 hold the sign-flipped variants for the
    split units).  start=True clears has_written BANK-wide, so only
    the FIRST matmul touching a (stream, bank) carries it - that also
    clears the HAM-warmup garbage (no separate init pass).
  - HAM warmup: dummy matmuls (on possibly-uninitialized operands -
    results are discarded) keep the PE busy from engine boot through
    the input-DMA wait so its clock gate opens (1.2 -> 2.4 GHz).
  - Tail: plain PSUM->SBUF copies into a bf16 staging tile (host
    upcasts; well inside the error budget): ScalarE evacuates
    streams 0, 1 and the right half of 2; VectorE stream 3 then the
    left half of 2 (consecutive DVE PSUM ops chain fine - the big
    DRAIN overlaps the next op).  Output DMAs spread over queues:
    sync ships d0, d3; gpsimd d1; ScalarE HWDGE d2; the output rings
    are pre-warmed by a tiny dummy store at ~11us.  Host adds
    sum_d W[f] - sum_d x[p] + border constants and transposes
    stream-major i-major -> (N, F, H, W).  NOTE: ordering any
    per-stream quarter op BETWEEN the chunk-4 split parts (to fire
    its stream sem early) reproducibly HANGS the device - likely the
    cayman event-accel deadlock; quarters must stay at the very end.

kernel(x, W) accepts the FULL inputs and returns the FULL output.
"""

import numpy as np
import ml_dtypes

import concourse.bass as bass
from concourse import mybir
from concourse.bass_utils import run_bass_kernel_spmd

# Problem constants (hardcoded per harness rules)
N, C, H, W_SP = 16, 64, 14, 14
F = 64
KK = 3
PAD = 1
P = H * W_SP            # 196 output positions per image
POS = N * P             # 3136 total positions
D = C * KK * KK         # 576
N_CORES = 8
F_PER = F // N_CORES    # 8 filters per core
NFULL = 4               # full 128-row d-chunks (d 0:512)
FOLD_FD = POS // 2      # 1568: folded chunk free dim
FQ1 = 392               # first folded DMA piece (cols 0:392)
NSTREAM = 4             # TensorE column-tiling streams
QPOS = POS // NSTREAM   # 784 positions per stream (its private quarter)
BANKC = 512             # psum bank capacity in f32
RING_V = 8              # vector-produced tile ring
RING_S = 4              # scalar-produced tile ring
N_WARM = 7              # HAM-warmup dummy matmul rounds per stream

FP32 = mybir.dt.float32
BF16 = mybir.dt.bfloat16

WCOLS = (NFULL + 1) * F_PER  # 40 W columns (full: k*8+j; folded: 32+j)
# stat layout (bf16): [0:64) full blocks (8 per filter, col j = +/-2
# by that filter's engine), [64:128) folded TOP blocks (rows 0:64),
# [128:192) folded BOT blocks (rows 64:128), [192:200) always-zero
# block for the HAM-warmup dummies, [200:208) the +2 full-variant
# block for the DVE half of the split unit (filter 7 is ACT -> -2
# in ST_FULL, but its chunk-4 DVE half needs +2).
ST_FULL = 0
ST_TOP = 64
ST_BOT = 128
ST_ZERO = 192
ST_XTRA = 200
ST_XTRA2 = 208
ST_N = 216
ACT_F = (3, 7)          # ScalarE filters (folded + chunks 1..3)
SPLIT_V = 1176          # chunk-4 filters 5,7: DVE cols 0:1176, ACT rest
LAST_J = 6              # chunk-4 filter 6: quartered per stream
# i-major position ordering (p = i*224 + n*14 + j) makes the padding
# zeros of kh=0 rows (chunk d[0:128]) the FIRST 224 positions and of
# kh=2 rows (chunk d[384:512]) the LAST 224: those column ranges are
# skipped on-device (min(0-w,0) is a per-filter constant the host
# adds to the border blocks).
BORD = 224
CH_LO = {1: BORD, 2: 0, 3: 0, 4: 0}            # per-chunk valid cols
CH_HI = {1: POS, 2: POS, 3: POS, 4: POS - BORD}


def _pieces(k, a, b):
    """(stream, psum_lo, mov_lo, width) pieces for source cols [a,b).

    Full chunks (k>=1): position p = col; stream c covers positions
    [784c, 784c+784).  Folded chunk (k==0): cols 0:784 are positions
    0:1568 halves -> streams 0 (TOP rows) & 2 (BOT rows); cols
    784:1568 -> streams 1 & 3.  psum col = position within quarter.
    """
    out = []
    if k == 0:
        lo, hi = a, min(b, QPOS)
        if lo < hi:
            out += [(0, lo, lo - a, hi - lo), (2, lo, lo - a, hi - lo)]
        lo, hi = max(a, QPOS), b
        if lo < hi:
            out += [(1, lo - QPOS, lo - a, hi - lo),
                    (3, lo - QPOS, lo - a, hi - lo)]
    else:
        for c in range(NSTREAM):
            lo, hi = max(a, QPOS * c), min(b, QPOS * (c + 1))
            if lo < hi:
                out.append((c, lo - QPOS * c, lo - a, hi - lo))
    return out


def _sub_split(lo, width):
    """Split a psum col range at the 512 bank boundary."""
    segs = []
    hi = lo + width
    if lo < BANKC and hi > BANKC:
        segs.append((lo, BANKC - lo))
        segs.append((BANKC, hi - BANKC))
    else:
        segs.append((lo, width))
    return segs


def build_bass():
    nc = bass.Bass()

    w_ext = nc.declare_dram_parameter("wcols", [128, WCOLS], FP32,
                                      isOutput=False)
    xf_ext = nc.declare_dram_parameter("xfold", [128, FOLD_FD], BF16,
                                       isOutput=False)
    x_ext = nc.declare_dram_parameter("xcol", [NFULL, 128, POS], BF16,
                                      isOutput=False)
    out_ext = nc.declare_dram_parameter("out", [NSTREAM, F_PER, QPOS],
                                        BF16, isOutput=True)

    # SBUF
    w_sb = nc.alloc_sbuf_tensor("w_sb", [128, WCOLS], FP32)
    stat = nc.alloc_sbuf_tensor("stat", [128, ST_N], BF16)
    zmov = nc.alloc_sbuf_tensor("zmov", [128, BANKC], BF16)
    xfold = nc.alloc_sbuf_tensor("xfold_sb", [128, FOLD_FD], BF16)
    xch = [nc.alloc_sbuf_tensor(f"xc{k}", [128, POS], BF16)
           for k in range(NFULL)]
    vring = [nc.alloc_sbuf_tensor(f"vb{r}", [128, POS], BF16)
             for r in range(RING_V)]
    sring = [nc.alloc_sbuf_tensor(f"sb{r}", [128, POS], BF16)
             for r in range(RING_S)]
    osb = nc.alloc_sbuf_tensor("osb", [128, QPOS], BF16)

    # PSUM: 8 banks; stream c owns banks {2c, 2c+1} = cols
    # [1024c, 1024c+784).
    psum = nc.alloc_psum_tensor("ps", [128, 8 * BANKC], FP32)

    # ---- op list, in PE consumption order --------------------------
    ops = []

    def add_op(eng, j, k, s_c0, fd, xtra=False):
        ops.append(dict(eng=eng, j=j, k=k, s_c0=s_c0, fd=fd, xtra=xtra,
                        pieces=_pieces(k, s_c0, s_c0 + fd)))

    VF = [j for j in range(F_PER) if j not in ACT_F]
    # folded chunk, first piece (cols 0:392) then rest (392:1568)
    for (a, b) in ((0, FQ1), (FQ1, FOLD_FD)):
        for j in VF:
            add_op("v", j, 0, a, b - a)
        for j in ACT_F:
            add_op("s", j, 0, a, b - a)
    # full chunks 1..3: vector filters first, then scalar
    for k in (1, 2, 3):
        for j in VF:
            add_op("v", j, k, CH_LO[k], CH_HI[k] - CH_LO[k])
        for j in ACT_F:
            add_op("s", j, k, CH_LO[k], CH_HI[k] - CH_LO[k])
    # chunk 4: vector filters (minus LAST_J and split 5), scalar
    # filter 3, the asymmetric filter-5/7 splits (DVE 0:1176, ACT
    # rest), then the quartered LAST_J unit
    for j in [j for j in VF if j not in (LAST_J, 5)]:
        add_op("v", j, 4, 0, CH_HI[4])
    add_op("s", 3, 4, 0, CH_HI[4])
    add_op("v", 5, 4, 0, SPLIT_V)                    # +2 via ST_FULL
    add_op("v", 7, 4, 0, SPLIT_V, xtra=True)         # +2 via ST_XTRA
    add_op("s", 5, 4, SPLIT_V, CH_HI[4] - SPLIT_V, xtra=True)  # ST_XTRA2
    add_op("s", 7, 4, SPLIT_V, CH_HI[4] - SPLIT_V)   # -2 via ST_FULL
    for c in (0, 1, 3, 2):
        add_op("v", LAST_J, 4, QPOS * c,
               min(QPOS * (c + 1), CH_HI[4]) - QPOS * c)

    # producer indices
    nv = ns = 0
    for op in ops:
        if op["eng"] == "v":
            op["r"] = nv
            nv += 1
        else:
            op["r"] = ns
            ns += 1

    # last op touching each stream (for stsem + stop flags)
    last_for_stream = {}
    for idx, op in enumerate(ops):
        for (c, _pl, _ml, _wd) in op["pieces"]:
            last_for_stream[c] = idx

    def stat_block(op, c):
        if op["xtra"]:
            base = ST_XTRA if op["eng"] == "v" else ST_XTRA2
            return stat[:, base:base + 8]
        j = op["j"]
        if op["k"] == 0:
            base = ST_TOP if c < 2 else ST_BOT
        else:
            base = ST_FULL
        return stat[:, base + 8 * j:base + 8 * j + 8]

    with (
        nc.Block() as block,
        nc.semaphore("xw_sem") as xw_sem,
        nc.semaphore("xfq_sem") as xfq_sem,    # folded piece 1 (gp)
        nc.semaphore("xfr_sem") as xfr_sem,    # folded rest (sync)
        nc.semaphore("x0_sem") as x0_sem,
        nc.semaphore("x1_sem") as x1_sem,
        nc.semaphore("x2_sem") as x2_sem,
        nc.semaphore("x3_sem") as x3_sem,
        nc.semaphore("out_sem") as out_sem,
        nc.semaphore("stat_sem") as stat_sem,  # stat blocks written
        nc.semaphore("dve_sem") as dve_sem,
        nc.semaphore("actp_sem") as actp_sem,
        nc.semaphore("pe_v_sem") as pe_v_sem,
        nc.semaphore("pe_s_sem") as pe_s_sem,
        nc.semaphore("ev01_sem") as ev01_sem,  # ScalarE evacs done
        nc.semaphore("ev23_sem") as ev23_sem,  # VectorE evacs done
        nc.semaphore("warm_sem") as warm_sem,  # ring pre-warm stores
        nc.semaphore("st0_sem") as st0_sem,
        nc.semaphore("st1_sem") as st1_sem,
        nc.semaphore("st2_sem") as st2_sem,
        nc.semaphore("st3_sem") as st3_sem,
    ):
        xsem = {1: x0_sem, 2: x1_sem, 3: x2_sem, 4: x3_sem}
        stsem = [st0_sem, st1_sem, st2_sem, st3_sem]

        def chunk_wait(eng_q, op, seen):
            k = op["k"]
            if k == 0:
                sem = xfq_sem if op["s_c0"] < FQ1 else xfr_sem
                if ("fold", op["s_c0"] < FQ1) not in seen:
                    seen[("fold", op["s_c0"] < FQ1)] = True
                    eng_q.wait_ge(sem, 16)
            else:
                if k not in seen:
                    seen[k] = True
                    eng_q.wait_ge(xsem[k], 16)

        @block.sync
        def _(sync: bass.BassEngine):
            # input DMAs in consumption order (fold piece 1, fold
            # rest, chunks); W rides the scalar engine's ring.
            sync.dma_start(out=xfold[:, 0:FQ1], in_=xf_ext[:, 0:FQ1],
                           single_packet=True).then_inc(xfq_sem, 16)
            sync.dma_start(out=xfold[:, FQ1:FOLD_FD],
                           in_=xf_ext[:, FQ1:FOLD_FD],
                           single_packet=True).then_inc(xfr_sem, 16)
            for k in range(NFULL):
                lo, hi = CH_LO[k + 1], CH_HI[k + 1]
                sync.dma_start(out=xch[k][:, lo:hi],
                               in_=x_ext[k][:, lo:hi],
                               single_packet=True).then_inc(xsem[k + 1], 16)
            # pre-warm the output ring (tiny store, overwritten later)
            sync.dma_start(out=out_ext[0][0:1, 0:2],
                           in_=osb[0:1, 0:2]).then_inc(warm_sem, 16)
            # output stores: d0, d1 (ScalarE evacs), d3 (VectorE evac)
            # fresh ring warm just before the tail (the boot-time warm
            # is ~27us stale by output time; descriptor-fetch latency
            # may be recency-sensitive)
            sync.wait_ge(dve_sem, 32)
            sync.dma_start(out=out_ext[0][0:1, 4:6],
                           in_=osb[0:1, 4:6]).then_inc(warm_sem, 16)
            sync.wait_ge(ev01_sem, 1)
            sync.dma_start(out=out_ext[0], in_=osb[0:F_PER, :],
                           single_packet=True).then_inc(out_sem, 16)
            sync.wait_ge(ev23_sem, 1)
            sync.dma_start(out=out_ext[3], in_=osb[96:96 + F_PER, :],
                           single_packet=True).then_inc(out_sem, 16)
            sync.wait_ge(out_sem, 16 * NSTREAM)

        @block.vector
        def _(vector: bass.BassEngine):
            vector.wait_ge(xw_sem, 16)
            seen = {}
            for op in ops:
                if op["eng"] != "v":
                    continue
                r = op["r"]
                chunk_wait(vector, op, seen)
                if r >= RING_V:
                    vector.wait_ge(pe_v_sem, r - RING_V + 1)
                col = (32 + op["j"]) if op["k"] == 0 \
                    else ((op["k"] - 1) * F_PER + op["j"])
                src = xfold if op["k"] == 0 else xch[op["k"] - 1]
                c0 = op["s_c0"]
                vector.tensor_scalar(
                    out=vring[r % RING_V][:, 0:op["fd"]],
                    in0=src[:, c0:c0 + op["fd"]],
                    scalar1=w_sb[:, col:col + 1], scalar2=0.0,
                    op0=mybir.AluOpType.subtract,
                    op1=mybir.AluOpType.min,
                ).then_inc(dve_sem, 1)
            # evacuate stream 3, then the left half of stream 2 (the
            # right half runs in parallel on ScalarE)
            vector.wait_ge(stsem[3], 1)
            vector.tensor_scalar(
                out=osb[96:96 + F_PER, :],
                in0=psum[96:96 + F_PER, 3072:3072 + QPOS],
                scalar1=0.0, scalar2=None,
                op0=mybir.AluOpType.add,
            ).then_inc(ev23_sem, 1)
            vector.wait_ge(stsem[2], 1)
            vector.tensor_scalar(
                out=osb[64:64 + F_PER, 0:392],
                in0=psum[64:64 + F_PER, 2048:2048 + 392],
                scalar1=0.0, scalar2=None,
                op0=mybir.AluOpType.add,
            ).then_inc(ev23_sem, 1)

        @block.scalar
        def _(scalar: bass.BassEngine):
            # W columns ride this engine's own DMA ring (parallel
            # latency with the sync ring's folded-x pieces)
            scalar.dma_start(out=w_sb[:], in_=w_ext[:],
                             single_packet=True).then_inc(xw_sem, 16)
            # touch the Relu table so the one-time ACT table load
            # overlaps the input DMAs instead of the first real unit
            scalar.activation(osb[0:1, 0:1], zmov[0:1, 0:1],
                              mybir.ActivationFunctionType.Relu,
                              bias=0.0, scale=1.0)
            scalar.wait_ge(xw_sem, 16)
            seen = {}
            for op in ops:
                if op["eng"] != "s":
                    continue
                r = op["r"]
                chunk_wait(scalar, op, seen)
                if r >= RING_S:
                    scalar.wait_ge(pe_s_sem, r - RING_S + 1)
                col = (32 + op["j"]) if op["k"] == 0 \
                    else ((op["k"] - 1) * F_PER + op["j"])
                src = xfold if op["k"] == 0 else xch[op["k"] - 1]
                c0 = op["s_c0"]
                scalar.activation(
                    sring[r % RING_S][:, 0:op["fd"]],
                    src[:, c0:c0 + op["fd"]],
                    mybir.ActivationFunctionType.Relu,
                    bias=w_sb[:, col:col + 1], scale=-1.0,
                ).then_inc(actp_sem, 1)
            # evacuate streams 0, 1 (st0 fires early thanks to the
            # q0-before-parts ordering), then the right half of
            # stream 2 (left half runs in parallel on VectorE), then
            # ship d2 from this queue
            for c in (0, 1):
                scalar.wait_ge(stsem[c], 1)
                scalar.activation(
                    osb[32 * c:32 * c + F_PER, :],
                    psum[32 * c:32 * c + F_PER, 1024 * c:1024 * c + QPOS],
                    mybir.ActivationFunctionType.Identity,
                    bias=0.0, scale=1.0,
                ).then_inc(ev01_sem, 1)
            scalar.wait_ge(stsem[2], 1)
            scalar.activation(
                osb[64:64 + F_PER, 392:QPOS],
                psum[64:64 + F_PER, 2048 + 392:2048 + QPOS],
                mybir.ActivationFunctionType.Identity,
                bias=0.0, scale=1.0,
            )
            scalar.wait_ge(ev23_sem, 2)   # DVE's left half done
            scalar.dma_start(out=out_ext[2], in_=osb[64:64 + F_PER, :],
                             single_packet=True).then_inc(out_sem, 16)

        @block.gpsimd
        def _(gp: bass.BassEngine):
            # one-time memsets (stationary blocks); warmup no longer
            # depends on them - it reads garbage and is discarded
            gp.memset(stat[:], 0.0)
            for j in range(F_PER):
                val = -2.0 if j in ACT_F else 2.0
                gp.memset(stat[:, ST_FULL + 8 * j + j:
                               ST_FULL + 8 * j + j + 1], val)
                gp.memset(stat[0:64, ST_TOP + 8 * j + j:
                               ST_TOP + 8 * j + j + 1], val)
                gp.memset(stat[64:128, ST_BOT + 8 * j + j:
                               ST_BOT + 8 * j + j + 1], val)
            # +2 full-variant block for filter 7's DVE split part and
            # -2 full-variant block for filter 5's ACT split part
            gp.memset(stat[:, ST_XTRA + 7:ST_XTRA + 8], 2.0)
            gp.memset(stat[:, ST_XTRA2 + 5:ST_XTRA2 + 6],
                      -2.0).then_inc(stat_sem, 1)
            # pre-warm this engine's output DMA ring
            gp.dma_start(out=out_ext[0][0:1, 2:4],
                         in_=osb[0:1, 2:4]).then_inc(warm_sem, 16)
            # fresh ring warm, then output store d1 (ScalarE's
            # second evacuation)
            gp.wait_ge(actp_sem, 11)
            gp.dma_start(out=out_ext[0][0:1, 6:8],
                         in_=osb[0:1, 6:8]).then_inc(warm_sem, 16)
            gp.wait_ge(ev01_sem, 2)
            gp.dma_start(out=out_ext[1], in_=osb[32:32 + F_PER, :],
                         single_packet=True).then_inc(out_sem, 16)

        @block.tensor
        def _(tensor: bass.BassEngine):
            # HAM warmup: keep the PE array busy through the input-DMA
            # wait so its clock gate opens (1.2 -> 2.4 GHz) before real
            # work.  Both operands may still be UNINITIALIZED - the
            # results are discarded (start=True'd by the first real
            # matmul into each region), so garbage is fine.
            for _w in range(N_WARM):
                for c in range(NSTREAM):
                    tensor.matmul(
                        psum[32 * c:32 * c + F_PER,
                             1024 * c:1024 * c + BANKC],
                        stat[:, ST_ZERO:ST_ZERO + 8],
                        zmov[:, 0:BANKC],
                        start=True, stop=True, skip_group_check=True,
                        tile_position=(0, 32 * c),
                    )
            tensor.wait_ge(stat_sem, 1)  # stationary blocks written
            started = set()
            for idx, op in enumerate(ops):
                kind, r = op["eng"], op["r"]
                if kind == "v":
                    tensor.wait_ge(dve_sem, r + 1)
                    a = vring[r % RING_V]
                else:
                    tensor.wait_ge(actp_sem, r + 1)
                    a = sring[r % RING_S]
                np_ = len(op["pieces"])
                for pi, (c, plo, mlo, wd) in enumerate(op["pieces"]):
                    blk = stat_block(op, c)
                    is_last_c = last_for_stream[c] == idx
                    segs = _sub_split(plo, wd)
                    for si, (so, sw) in enumerate(segs):
                        fin = pi == np_ - 1 and si == len(segs) - 1
                        # start=True clears has_written BANK-wide, so
                        # only the FIRST matmul touching a (stream,
                        # bank) carries it; later first-writes to other
                        # columns of that bank replace-on-write because
                        # the bank clear already reset their bits.
                        key = (c, 0 if so < BANKC else 1)
                        st = key not in started
                        started.add(key)
                        stop_f = is_last_c and si == len(segs) - 1
                        mm = tensor.matmul(
                            psum[32 * c:32 * c + F_PER,
                                 1024 * c + so:1024 * c + so + sw],
                            blk,
                            a[:, mlo + (so - plo):mlo + (so - plo) + sw],
                            start=st, stop=stop_f,
                            skip_group_check=True,
                            tile_position=(0, 32 * c),
                        )
                        if stop_f:
                            mm.then_inc(stsem[c], 1)
                        elif fin:
                            mm.then_inc(
                                pe_v_sem if kind == "v" else pe_s_sem, 1)

    return nc


def _prep_inputs(x: np.ndarray, W: np.ndarray):
    x = np.asarray(x, dtype=np.float32)
    W = np.asarray(W, dtype=np.float32)
    # Host im2col in (kh, kw, c) d-order, i-MAJOR positions
    # (p = i*224 + n*14 + j)
    xp = np.zeros((C, N, H + 2, W_SP + 2), np.float32)
    xp[:, :, PAD:PAD + H, PAD:PAD + W_SP] = x.transpose(1, 0, 2, 3)
    xc = np.zeros((D, POS), np.float32)
    for b in range(KK * KK):
        kh, kw = divmod(b, KK)
        xc[64 * b:64 * (b + 1), :] = (
            xp[:, :, kh:kh + H, kw:kw + W_SP]
            .transpose(0, 2, 1, 3).reshape(C, POS))
    xfull = (xc[:512].reshape(NFULL, 128, POS)).astype(ml_dtypes.bfloat16)
    xfoldb = xc[512:].astype(ml_dtypes.bfloat16)
    xfold = np.zeros((128, FOLD_FD), ml_dtypes.bfloat16)
    xfold[0:64, :] = xfoldb[:, 0:FOLD_FD]
    xfold[64:128, :] = xfoldb[:, FOLD_FD:POS]
    # -sum_d x over the bf16-ROUNDED values the device actually uses
    negx = -np.asarray(xfull, np.float32).sum(axis=(0, 1)) \
        - np.asarray(xfoldb, np.float32).sum(axis=0)
    # W_col in (kh, kw, c) d-order, bf16-rounded (shipped as fp32 so
    # tensor_scalar's scalar1 constraint is met, but the VALUES match
    # bf16 so device/host arithmetic agrees): (F, 576)
    Wp = W.transpose(0, 2, 3, 1).reshape(F, KK * KK * C)
    Wpb = np.asarray(Wp.astype(ml_dtypes.bfloat16), np.float32)
    sw = Wpb.sum(axis=1)  # (F,)
    wtiles = []
    for i in range(N_CORES):
        wt = np.zeros((128, WCOLS), np.float32)
        for k in range(NFULL):
            wt[:, k * F_PER:(k + 1) * F_PER] = \
                Wpb[F_PER * i:F_PER * (i + 1), 128 * k:128 * (k + 1)].T
        fb = Wpb[F_PER * i:F_PER * (i + 1), 512:D].T  # (64, 8)
        wt[0:64, 32:40] = fb
        wt[64:128, 32:40] = fb
        wtiles.append(wt)
    # skipped border contributions: 2*sum min(-w,0) over the skipped
    # d rows, added by the host to the border position blocks
    b0 = 2.0 * np.minimum(-Wpb[:, 0:128], 0.0).sum(axis=1)     # (F,)
    b13 = 2.0 * np.minimum(-Wpb[:, 384:512], 0.0).sum(axis=1)  # (F,)
    return xfull, xfold, wtiles, sw, negx, b0, b13


_CACHED_NC = None
LAST_RESULT = None  # BassKernelResults of the most recent run (for test.py)


def kernel(x: np.ndarray, W: np.ndarray, _trace: bool = False) -> np.ndarray:
    global _CACHED_NC, LAST_RESULT
    xfull, xfold, wtiles, sw, negx, b0, b13 = _prep_inputs(x, W)
    if _CACHED_NC is None:
        _CACHED_NC = build_bass()
    nc = _CACHED_NC
    in_maps = [{"xcol": xfull, "xfold": xfold, "wcols": wtiles[i]}
               for i in range(N_CORES)]
    res = run_bass_kernel_spmd(nc, in_maps, core_ids=list(range(N_CORES)),
                               trace=_trace)
    LAST_RESULT = res
    outs = [np.asarray(res.results[i]["out"], dtype=np.float32)
            .transpose(1, 0, 2).reshape(F_PER, POS)
            for i in range(N_CORES)]
    o = np.concatenate(outs, axis=0)                    # (64, 3136)
    # host finish: rank-1 identity terms + skipped border constants
    o = o + sw[:, None] + negx[None, :]
    o[:, 0:BORD] += b0[:, None]
    o[:, POS - BORD:POS] += b13[:, None]
    # i-major positions: p = i*224 + n*14 + j
    o = (o.reshape(F, H, N, W_SP).transpose(2, 0, 1, 3)
          .reshape(N, F, H, W_SP).astype(np.float32))
    return o


# revision 31
# speedup vs baseline: 1.0160x; 1.0107x over previous
"""AdderNet 2D conv (L1-distance "convolution") on 8 TRN2 NeuronCores.

Reference computation:
    X_col = unfold(x, k=3, stride=1, pad=1)      # (N, D, P)  D=576, P=196
    out[n, f, p] = -sum_d |W_col[f, d] - X_col[n, d, p]|

Distribution: filter-parallel - core i computes filters f in [8i, 8i+8)
for the FULL batch (no collectives; host concatenates filter slices).

Identity used on-device:
  -sum_d |x-w|  =  (-sum_d x) + (sum_d w) + 2*sum_d min(x-w, 0)
The rank-1 terms (-sum_d x per position, +sum_d w per filter) are
added ON THE HOST after the gather, so the device does ONLY the
min/relu elementwise stream + PE reduction.

Per-core pipeline (raw Bass):
  - Host im2col, d (patch dim, 576, (kh,kw,c)-ordered) on SBUF
    partitions: FOUR full 128-row chunks plus one FOLDED half-chunk
    (d 512:576 for positions 0:1568 on partitions 0:64 and positions
    1568:3136 on partitions 64:128 -> a (128, 1568) tile).
  - Positions are i-MAJOR (p = i*224 + n*14 + j), which makes the
    padding zeros of the kh=0 rows (chunk d 0:128) the FIRST 224
    positions and of the kh=2 rows (chunk d 384:512) the LAST 224;
    those column ranges are skipped on-device (about 5% less
    elementwise + DMA) and the host adds the constant
    2*sum min(-w,0) to the border position blocks.
  - Input DMAs: W columns (fp32, values pre-rounded to bf16 so
    device and host arithmetic agree) ride the SCALAR engine's own
    DMA ring (parallel latency); the sync ring carries, in
    consumption order, the folded tile as a 392-col piece + rest
    (first elementwise op starts ~9.9us), then chunks 1..4 with the
    skipped border columns trimmed.  NOT the gpsimd queue: its SWDGE
    path measured ~5us issue-to-data vs ~2us for hardware rings.
  - Per (filter, chunk) unit, ONE elementwise instruction:
      VectorE: tensor_scalar(op0=sub W[f,.], op1=min 0) -> min(x-w,0)
        (4x_2p DVE perf mode; ~946ns/full 3136-col unit)
      ScalarE: activation(Relu, scale=-1, bias=W[f,.])  -> relu(w-x)
        (1x rate, ~2.9us/full unit; capped by HW at 1 elem/cyc/lane)
    Split by measured rates: ScalarE gets filters {3,7} in the folded
    chunk and chunks 1-3, filter 3 in chunk 4, plus cols 1176:2912 of
    the chunk-4 filter-5/7 units (VectorE makes cols 0:1176).  The
    LAST unit (filter 6, chunk 4) is emitted as per-stream quarter
    ops in order (0,1,3,2) so stream completion cascades into the
    output path.  GPSIMD is useless here: its tensor_scalar measured
    45x slower than DVE and TT-min / scalar_tensor_tensor fail the
    Pool-engine codegen check.
  - TensorE reduces over partitions into PSUM, 4-way column-tiled:
    stream c (tile_position=(0,32c)) computes ALL 8 filters for its
    private position quarter [784c, 784c+784), psum rows 32c..32c+8,
    banks {2c, 2c+1}.  Stationary blocks: +/-2 (sign by producing
    engine; ST_XTRA/ST_XTRA2 hold the sign-flipped variants for the
    split units).  start=True clears has_written BANK-wide, so only
    the FIRST matmul touching a (stream, bank) carries it - that also
    clears the HAM-warmup garbage (no separate init pass).
  - HAM warmup: dummy matmuls (on possibly-uninitialized operands -
    results are discarded) keep the PE busy from engine boot through
    the input-DMA wait so its clock gate opens (1.2 -> 2.4 GHz).
  - Tail: plain PSUM->SBUF copies into a bf16 staging tile (host
    upcasts; well inside the error budget): ScalarE evacuates
    streams 0, 1 and the right half of 2; VectorE stream 3 then the
    left half of 2 (consecutive DVE PSUM ops chain fine - the big
    DRAIN overlaps the next op).  Output DMAs spread over queues:
    sync ships d0, d3; gpsimd d1; ScalarE HWDGE d2; the output rings
    are pre-warmed by a tiny dummy store at ~11us.  Host adds
    sum_d W[f] - sum_d x[p] + border constants and transposes
    stream-major i-major -> (N, F, H, W).  NOTE: ordering any
    per-stream quarter op BETWEEN the chunk-4 split parts (to fire
    its stream sem early) reproducibly HANGS the device - likely the
    cayman event-accel deadlock; quarters must stay at the very end.

kernel(x, W) accepts the FULL inputs and returns the FULL output.
"""

import numpy as np
import ml_dtypes

import concourse.bass as bass
from concourse import mybir
from concourse.bass_utils import run_bass_kernel_spmd

# Problem constants (hardcoded per harness rules)
N, C, H, W_SP = 16, 64, 14, 14
F = 64
KK = 3
PAD = 1
P = H * W_SP            # 196 output positions per image
POS = N * P             # 3136 total positions
D = C * KK * KK         # 576
N_CORES = 8
F_PER = F // N_CORES    # 8 filters per core
NFULL = 4               # full 128-row d-chunks (d 0:512)
FOLD_FD = POS // 2      # 1568: folded chunk free dim
FQ1 = 392               # first folded DMA piece (cols 0:392)
NSTREAM = 4             # TensorE column-tiling streams
QPOS = POS // NSTREAM   # 784 positions per stream (its private quarter)
BANKC = 512             # psum bank capacity in f32
RING_V = 8              # vector-produced tile ring
RING_S = 4              # scalar-produced tile ring
N_WARM = 7              # HAM-warmup dummy matmul rounds per stream

FP32 = mybir.dt.float32
BF16 = mybir.dt.bfloat16

WCOLS = (NFULL + 1) * F_PER  # 40 W columns (full: k*8+j; folded: 32+j)
# stat layout (bf16): [0:64) full blocks (8 per filter, col j = +/-2
# by that filter's engine), [64:128) folded TOP blocks (rows 0:64),
# [128:192) folded BOT blocks (rows 64:128), [192:200) always-zero
# block for the HAM-warmup dummies, [200:208) the +2 full-variant
# block for the DVE half of the split unit (filter 7 is ACT -> -2
# in ST_FULL, but its chunk-4 DVE half needs +2).
ST_FULL = 0
ST_TOP = 64
ST_BOT = 128
ST_ZERO = 192
ST_XTRA = 200
ST_XTRA2 = 208
ST_N = 216
ACT_F = (3, 7)          # ScalarE filters (folded + chunks 1..3)
SPLIT_V = 1176          # chunk-4 filters 5,7: DVE cols 0:1176, ACT rest
LAST_J = 6              # chunk-4 filter 6: quartered per stream
# i-major position ordering (p = i*224 + n*14 + j) makes the padding
# zeros of kh=0 rows (chunk d[0:128]) the FIRST 224 positions and of
# kh=2 rows (chunk d[384:512]) the LAST 224: those column ranges are
# skipped on-device (min(0-w,0) is a per-filter constant the host
# adds to the border blocks).
BORD = 224
CH_LO = {1: BORD, 2: 0, 3: 0, 4: 0}            # per-chunk valid cols
CH_HI = {1: POS, 2: POS, 3: POS, 4: POS - BORD}


def _pieces(k, a, b):
    """(stream, psum_lo, mov_lo, width) pieces for source cols [a,b).

    Full chunks (k>=1): position p = col; stream c covers positions
    [784c, 784c+784).  Folded chunk (k==0): cols 0:784 are positions
    0:1568 halves -> streams 0 (TOP rows) & 2 (BOT rows); cols
    784:1568 -> streams 1 & 3.  psum col = position within quarter.
    """
    out = []
    if k == 0:
        lo, hi = a, min(b, QPOS)
        if lo < hi:
            out += [(0, lo, lo - a, hi - lo), (2, lo, lo - a, hi - lo)]
        lo, hi = max(a, QPOS), b
        if lo < hi:
            out += [(1, lo - QPOS, lo - a, hi - lo),
                    (3, lo - QPOS, lo - a, hi - lo)]
    else:
        for c in range(NSTREAM):
            lo, hi = max(a, QPOS * c), min(b, QPOS * (c + 1))
            if lo < hi:
                out.append((c, lo - QPOS * c, lo - a, hi - lo))
    return out


def _sub_split(lo, width):
    """Split a psum col range at the 512 bank boundary."""
    segs = []
    hi = lo + width
    if lo < BANKC and hi > BANKC:
        segs.append((lo, BANKC - lo))
        segs.append((BANKC, hi - BANKC))
    else:
        segs.append((lo, width))
    return segs


def build_bass():
    nc = bass.Bass()

    w_ext = nc.declare_dram_parameter("wcols", [128, WCOLS], FP32,
                                      isOutput=False)
    xf_ext = nc.declare_dram_parameter("xfold", [128, FOLD_FD], BF16,
                                       isOutput=False)
    x_ext = nc.declare_dram_parameter("xcol", [NFULL, 128, POS], BF16,
                                      isOutput=False)
    out_ext = nc.declare_dram_parameter("out", [NSTREAM, F_PER, QPOS],
                                        BF16, isOutput=True)

    # SBUF
    w_sb = nc.alloc_sbuf_tensor("w_sb", [128, WCOLS], FP32)
    stat = nc.alloc_sbuf_tensor("stat", [128, ST_N], BF16)
    zmov = nc.alloc_sbuf_tensor("zmov", [128, BANKC], BF16)
    xfold = nc.alloc_sbuf_tensor("xfold_sb", [128, FOLD_FD], BF16)
    xch = [nc.alloc_sbuf_tensor(f"xc{k}", [128, POS], BF16)
           for k in range(NFULL)]
    vring = [nc.alloc_sbuf_tensor(f"vb{r}", [128, POS], BF16)
             for r in range(RING_V)]
    sring = [nc.alloc_sbuf_tensor(f"sb{r}", [128, POS], BF16)
             for r in range(RING_S)]
    osb = nc.alloc_sbuf_tensor("osb", [128, QPOS], BF16)

    # PSUM: 8 banks; stream c owns banks {2c, 2c+1} = cols
    # [1024c, 1024c+784).
    psum = nc.alloc_psum_tensor("ps", [128, 8 * BANKC], FP32)

    # ---- op list, in PE consumption order --------------------------
    ops = []

    def add_op(eng, j, k, s_c0, fd, xtra=False):
        ops.append(dict(eng=eng, j=j, k=k, s_c0=s_c0, fd=fd, xtra=xtra,
                        pieces=_pieces(k, s_c0, s_c0 + fd)))

    VF = [j for j in range(F_PER) if j not in ACT_F]
    # folded chunk, first piece (cols 0:392) then rest (392:1568)
    for (a, b) in ((0, FQ1), (FQ1, FOLD_FD)):
        for j in VF:
            add_op("v", j, 0, a, b - a)
        for j in ACT_F:
            add_op("s", j, 0, a, b - a)
    # full chunks 1..3: vector filters first, then scalar
    for k in (1, 2, 3):
        for j in VF:
            add_op("v", j, k, CH_LO[k], CH_HI[k] - CH_LO[k])
        for j in ACT_F:
            add_op("s", j, k, CH_LO[k], CH_HI[k] - CH_LO[k])
    # chunk 4: vector filters (minus LAST_J and split 5), scalar
    # filter 3, the asymmetric filter-5/7 splits (DVE 0:1176, ACT
    # rest), then the quartered LAST_J unit
    for j in [j for j in VF if j not in (LAST_J, 5)]:
        add_op("v", j, 4, 0, CH_HI[4])
    add_op("s", 3, 4, 0, CH_HI[4])
    add_op("v", 5, 4, 0, SPLIT_V)                    # +2 via ST_FULL
    add_op("v", 7, 4, 0, SPLIT_V, xtra=True)         # +2 via ST_XTRA
    add_op("s", 5, 4, SPLIT_V, CH_HI[4] - SPLIT_V, xtra=True)  # ST_XTRA2
    add_op("s", 7, 4, SPLIT_V, CH_HI[4] - SPLIT_V)   # -2 via ST_FULL
    for c in (0, 1, 3, 2):
        add_op("v", LAST_J, 4, QPOS * c,
               min(QPOS * (c + 1), CH_HI[4]) - QPOS * c)

    # producer indices
    nv = ns = 0
    for op in ops:
        if op["eng"] == "v":
            op["r"] = nv
            nv += 1
        else:
            op["r"] = ns
            ns += 1

    # last op touching each stream (for stsem + stop flags)
    last_for_stream = {}
    for idx, op in enumerate(ops):
        for (c, _pl, _ml, _wd) in op["pieces"]:
            last_for_stream[c] = idx

    def stat_block(op, c):
        if op["xtra"]:
            base = ST_XTRA if op["eng"] == "v" else ST_XTRA2
            return stat[:, base:base + 8]
        j = op["j"]
        if op["k"] == 0:
            base = ST_TOP if c < 2 else ST_BOT
        else:
            base = ST_FULL
        return stat[:, base + 8 * j:base + 8 * j + 8]

    with (
        nc.Block() as block,
        nc.semaphore("xw_sem") as xw_sem,
        nc.semaphore("xfq_sem") as xfq_sem,    # folded piece 1 (gp)
        nc.semaphore("xfr_sem") as xfr_sem,    # folded rest (sync)
        nc.semaphore("x0_sem") as x0_sem,
        nc.semaphore("x1_sem") as x1_sem,
        nc.semaphore("x2_sem") as x2_sem,
        nc.semaphore("x3_sem") as x3_sem,
        nc.semaphore("out_sem") as out_sem,
        nc.semaphore("stat_sem") as stat_sem,  # stat blocks written
        nc.semaphore("dve_sem") as dve_sem,
        nc.semaphore("actp_sem") as actp_sem,
        nc.semaphore("pe_v_sem") as pe_v_sem,
        nc.semaphore("pe_s_sem") as pe_s_sem,
        nc.semaphore("ev01_sem") as ev01_sem,  # ScalarE evacs done
        nc.semaphore("ev23_sem") as ev23_sem,  # VectorE evacs done
        nc.semaphore("warm_sem") as warm_sem,  # ring pre-warm stores
        nc.semaphore("st0_sem") as st0_sem,
        nc.semaphore("st1_sem") as st1_sem,
        nc.semaphore("st2_sem") as st2_sem,
        nc.semaphore("st3_sem") as st3_sem,
    ):
        xsem = {1: x0_sem, 2: x1_sem, 3: x2_sem, 4: x3_sem}
        stsem = [st0_sem, st1_sem, st2_sem, st3_sem]

        def chunk_wait(eng_q, op, seen):
            k = op["k"]
            if k == 0:
                sem = xfq_sem if op["s_c0"] < FQ1 else xfr_sem
                if ("fold", op["s_c0"] < FQ1) not in seen:
                    seen[("fold", op["s_c0"] < FQ1)] = True
                    eng_q.wait_ge(sem, 16)
            else:
                if k not in seen:
                    seen[k] = True
                    eng_q.wait_ge(xsem[k], 16)

        @block.sync
        def _(sync: bass.BassEngine):
            # input DMAs in consumption order (fold piece 1, fold
            # rest, chunks); W rides the scalar engine's ring.
            sync.dma_start(out=xfold[:, 0:FQ1], in_=xf_ext[:, 0:FQ1],
                           single_packet=True).then_inc(xfq_sem, 16)
            sync.dma_start(out=xfold[:, FQ1:FOLD_FD],
                           in_=xf_ext[:, FQ1:FOLD_FD],
                           single_packet=True).then_inc(xfr_sem, 16)
            for k in range(NFULL):
                lo, hi = CH_LO[k + 1], CH_HI[k + 1]
                sync.dma_start(out=xch[k][:, lo:hi],
                               in_=x_ext[k][:, lo:hi],
                               single_packet=True).then_inc(xsem[k + 1], 16)
            # pre-warm the output ring (tiny store, overwritten later)
            sync.dma_start(out=out_ext[0][0:1, 0:2],
                           in_=osb[0:1, 0:2]).then_inc(warm_sem, 16)
            # output stores: d0, d1 (ScalarE evacs), d3 (VectorE evac)
            sync.wait_ge(ev01_sem, 1)
            sync.dma_start(out=out_ext[0], in_=osb[0:F_PER, :],
                           single_packet=True).then_inc(out_sem, 16)
            sync.wait_ge(ev23_sem, 1)
            sync.dma_start(out=out_ext[3], in_=osb[96:96 + F_PER, :],
                           single_packet=True).then_inc(out_sem, 16)
            sync.wait_ge(out_sem, 16 * NSTREAM)

        @block.vector
        def _(vector: bass.BassEngine):
            vector.wait_ge(xw_sem, 16)
            seen = {}
            for op in ops:
                if op["eng"] != "v":
                    continue
                r = op["r"]
                chunk_wait(vector, op, seen)
                if r >= RING_V:
                    vector.wait_ge(pe_v_sem, r - RING_V + 1)
                col = (32 + op["j"]) if op["k"] == 0 \
                    else ((op["k"] - 1) * F_PER + op["j"])
                src = xfold if op["k"] == 0 else xch[op["k"] - 1]
                c0 = op["s_c0"]
                vector.tensor_scalar(
                    out=vring[r % RING_V][:, 0:op["fd"]],
                    in0=src[:, c0:c0 + op["fd"]],
                    scalar1=w_sb[:, col:col + 1], scalar2=0.0,
                    op0=mybir.AluOpType.subtract,
                    op1=mybir.AluOpType.min,
                ).then_inc(dve_sem, 1)
            # evacuate stream 3, then the left half of stream 2 (the
            # right half runs in parallel on ScalarE)
            vector.wait_ge(stsem[3], 1)
            vector.tensor_scalar(
                out=osb[96:96 + F_PER, :],
                in0=psum[96:96 + F_PER, 3072:3072 + QPOS],
                scalar1=0.0, scalar2=None,
                op0=mybir.AluOpType.add,
            ).then_inc(ev23_sem, 1)
            vector.wait_ge(stsem[2], 1)
            vector.tensor_scalar(
                out=osb[64:64 + F_PER, 0:392],
                in0=psum[64:64 + F_PER, 2048:2048 + 392],
                scalar1=0.0, scalar2=None,
                op0=mybir.AluOpType.add,
            ).then_inc(ev23_sem, 1)

        @block.scalar
        def _(scalar: bass.BassEngine):
            # W columns ride this engine's own DMA ring (parallel
            # latency with the sync ring's folded-x pieces)
            scalar.dma_start(out=w_sb[:], in_=w_ext[:],
                             single_packet=True).then_inc(xw_sem, 16)
            # touch the Relu table so the one-time ACT table load
            # overlaps the input DMAs instead of the first real unit
            scalar.activation(osb[0:1, 0:1], zmov[0:1, 0:1],
                              mybir.ActivationFunctionType.Relu,
                              bias=0.0, scale=1.0)
            scalar.wait_ge(xw_sem, 16)
            seen = {}
            for op in ops:
                if op["eng"] != "s":
                    continue
                r = op["r"]
                chunk_wait(scalar, op, seen)
                if r >= RING_S:
                    scalar.wait_ge(pe_s_sem, r - RING_S + 1)
                col = (32 + op["j"]) if op["k"] == 0 \
                    else ((op["k"] - 1) * F_PER + op["j"])
                src = xfold if op["k"] == 0 else xch[op["k"] - 1]
                c0 = op["s_c0"]
                scalar.activation(
                    sring[r % RING_S][:, 0:op["fd"]],
                    src[:, c0:c0 + op["fd"]],
                    mybir.ActivationFunctionType.Relu,
                    bias=w_sb[:, col:col + 1], scale=-1.0,
                ).then_inc(actp_sem, 1)
            # evacuate streams 0, 1 (st0 fires early thanks to the
            # q0-before-parts ordering), then the right half of
            # stream 2 (left half runs in parallel on VectorE), then
            # ship d2 from this queue
            for c in (0, 1):
                scalar.wait_ge(stsem[c], 1)
                scalar.activation(
                    osb[32 * c:32 * c + F_PER, :],
                    psum[32 * c:32 * c + F_PER, 1024 * c:1024 * c + QPOS],
                    mybir.ActivationFunctionType.Identity,
                    bias=0.0, scale=1.0,
                ).then_inc(ev01_sem, 1)
            scalar.wait_ge(stsem[2], 1)
            scalar.activation(
                osb[64:64 + F_PER, 392:QPOS],
                psum[64:64 + F_PER, 2048 + 392:2048 + QPOS],
                mybir.ActivationFunctionType.Identity,
                bias=0.0, scale=1.0,
            )
            scalar.wait_ge(ev23_sem, 2)   # DVE's left half done
            scalar.dma_start(out=out_ext[2], in_=osb[64:64 + F_PER, :],
                             single_packet=True).then_inc(out_sem, 16)

        @block.gpsimd
        def _(gp: bass.BassEngine):
            # one-time memsets (stationary blocks); warmup no longer
            # depends on them - it reads garbage and is discarded
            gp.memset(stat[:], 0.0)
            for j in range(F_PER):
                val = -2.0 if j in ACT_F else 2.0
                gp.memset(stat[:, ST_FULL + 8 * j + j:
                               ST_FULL + 8 * j + j + 1], val)
                gp.memset(stat[0:64, ST_TOP + 8 * j + j:
                               ST_TOP + 8 * j + j + 1], val)
                gp.memset(stat[64:128, ST_BOT + 8 * j + j:
                               ST_BOT + 8 * j + j + 1], val)
            # +2 full-variant block for filter 7's DVE split part and
            # -2 full-variant block for filter 5's ACT split part
            gp.memset(stat[:, ST_XTRA + 7:ST_XTRA + 8], 2.0)
            gp.memset(stat[:, ST_XTRA2 + 5:ST_XTRA2 + 6],
                      -2.0).then_inc(stat_sem, 1)
            # pre-warm this engine's output DMA ring
            gp.dma_start(out=out_ext[0][0:1, 2:4],
                         in_=osb[0:1, 2:4]).then_inc(warm_sem, 16)
            # output store d1 (ScalarE's second evacuation)
            gp.wait_ge(ev01_sem, 2)
            gp.dma_start(out=out_ext[1], in_=osb[32:32 + F_PER, :],
                         single_packet=True).then_inc(out_sem, 16)

        @block.tensor
        def _(tensor: bass.BassEngine):
            # HAM warmup: keep the PE array busy through the input-DMA
            # wait so its clock gate opens (1.2 -> 2.4 GHz) before real
            # work.  Both operands may still be UNINITIALIZED - the
            # results are discarded (start=True'd by the first real
            # matmul into each region), so garbage is fine.
            for _w in range(N_WARM):
                for c in range(NSTREAM):
                    tensor.matmul(
                        psum[32 * c:32 * c + F_PER,
                             1024 * c:1024 * c + BANKC],
                        stat[:, ST_ZERO:ST_ZERO + 8],
                        zmov[:, 0:BANKC],
                        start=True, stop=True, skip_group_check=True,
                        tile_position=(0, 32 * c),
                    )
            tensor.wait_ge(stat_sem, 1)  # stationary blocks written
            started = set()
            for idx, op in enumerate(ops):
                kind, r = op["eng"], op["r"]
                if kind == "v":
                    tensor.wait_ge(dve_sem, r + 1)
                    a = vring[r % RING_V]
                else:
                    tensor.wait_ge(actp_sem, r + 1)
                    a = sring[r % RING_S]
                np_ = len(op["pieces"])
                for pi, (c, plo, mlo, wd) in enumerate(op["pieces"]):
                    blk = stat_block(op, c)
                    is_last_c = last_for_stream[c] == idx
                    segs = _sub_split(plo, wd)
                    for si, (so, sw) in enumerate(segs):
                        fin = pi == np_ - 1 and si == len(segs) - 1
                        # start=True clears has_written BANK-wide, so
                        # only the FIRST matmul touching a (stream,
                        # bank) carries it; later first-writes to other
                        # columns of that bank replace-on-write because
                        # the bank clear already reset their bits.
                        key = (c, 0 if so < BANKC else 1)
                        st = key not in started
                        started.add(key)
                        stop_f = is_last_c and si == len(segs) - 1
                        mm = tensor.matmul(
                            psum[32 * c:32 * c + F_PER,
                                 1024 * c + so:1024 * c + so + sw],
                            blk,
                            a[:, mlo + (so - plo):mlo + (so - plo) + sw],
                            start=st, stop=stop_f,
                            skip_group_check=True,
                            tile_position=(0, 32 * c),
                        )
                        if stop_f:
                            mm.then_inc(stsem[c], 1)
                        elif fin:
                            mm.then_inc(
                                pe_v_sem if kind == "v" else pe_s_sem, 1)

    return nc


def _prep_inputs(x: np.ndarray, W: np.ndarray):
    x = np.asarray(x, dtype=np.float32)
    W = np.asarray(W, dtype=np.float32)
    # Host im2col in (kh, kw, c) d-order, i-MAJOR positions
    # (p = i*224 + n*14 + j)
    xp = np.zeros((C, N, H + 2, W_SP + 2), np.float32)
    xp[:, :, PAD:PAD + H, PAD:PAD + W_SP] = x.transpose(1, 0, 2, 3)
    xc = np.zeros((D, POS), np.float32)
    for b in range(KK * KK):
        kh, kw = divmod(b, KK)
        xc[64 * b:64 * (b + 1), :] = (
            xp[:, :, kh:kh + H, kw:kw + W_SP]
            .transpose(0, 2, 1, 3).reshape(C, POS))
    xfull = (xc[:512].reshape(NFULL, 128, POS)).astype(ml_dtypes.bfloat16)
    xfoldb = xc[512:].astype(ml_dtypes.bfloat16)
    xfold = np.zeros((128, FOLD_FD), ml_dtypes.bfloat16)
    xfold[0:64, :] = xfoldb[:, 0:FOLD_FD]
    xfold[64:128, :] = xfoldb[:, FOLD_FD:POS]
    # -sum_d x over the bf16-ROUNDED values the device actually uses
    negx = -np.asarray(xfull, np.float32).sum(axis=(0, 1)) \
        - np.asarray(xfoldb, np.float32).sum(axis=0)
    # W_col in (kh, kw, c) d-order, bf16-rounded (shipped as fp32 so
    # tensor_scalar's scalar1 constraint is met, but the VALUES match
    # bf16 so device/host arithmetic agrees): (F, 576)
    Wp = W.transpose(0, 2, 3, 1).reshape(F, KK * KK * C)
    Wpb = np.asarray(Wp.astype(ml_dtypes.bfloat16), np.float32)
    sw = Wpb.sum(axis=1)  # (F,)
    wtiles = []
    for i in range(N_CORES):
        wt = np.zeros((128, WCOLS), np.float32)
        for k in range(NFULL):
            wt[:, k * F_PER:(k + 1) * F_PER] = \
                Wpb[F_PER * i:F_PER * (i + 1), 128 * k:128 * (k + 1)].T
        fb = Wpb[F_PER * i:F_PER * (i + 1), 512:D].T  # (64, 8)
        wt[0:64, 32:40] = fb
        wt[64:128, 32:40] = fb
        wtiles.append(wt)
    # skipped border contributions: 2*sum min(-w,0) over the skipped
    # d rows, added by the host to the border position blocks
    b0 = 2.0 * np.minimum(-Wpb[:, 0:128], 0.0).sum(axis=1)     # (F,)
    b13 = 2.0 * np.minimum(-Wpb[:, 384:512], 0.0).sum(axis=1)  # (F,)
    return xfull, xfold, wtiles, sw, negx, b0, b13


_CACHED_NC = None
LAST_RESULT = None  # BassKernelResults of the most recent run (for test.py)


def kernel(x: np.ndarray, W: np.ndarray, _trace: bool = False) -> np.ndarray:
    global _CACHED_NC, LAST_RESULT
    xfull, xfold, wtiles, sw, negx, b0, b13 = _prep_inputs(x, W)
    if _CACHED_NC is None:
        _CACHED_NC = build_bass()
    nc = _CACHED_NC
    in_maps = [{"xcol": xfull, "xfold": xfold, "wcols": wtiles[i]}
               for i in range(N_CORES)]
    res = run_bass_kernel_spmd(nc, in_maps, core_ids=list(range(N_CORES)),
                               trace=_trace)
    LAST_RESULT = res
    outs = [np.asarray(res.results[i]["out"], dtype=np.float32)
            .transpose(1, 0, 2).reshape(F_PER, POS)
            for i in range(N_CORES)]
    o = np.concatenate(outs, axis=0)                    # (64, 3136)
    # host finish: rank-1 identity terms + skipped border constants
    o = o + sw[:, None] + negx[None, :]
    o[:, 0:BORD] += b0[:, None]
    o[:, POS - BORD:POS] += b13[:, None]
    # i-major positions: p = i*224 + n*14 + j
    o = (o.reshape(F, H, N, W_SP).transpose(2, 0, 1, 3)
          .reshape(N, F, H, W_SP).astype(np.float32))
    return o
